# revision 1
# baseline (speedup 1.0000x reference)
"""Dense transformer block on 8 Trainium2 NeuronCores.

Sharding: each core owns half a batch element (512 rows out of [4, 1024, C]).
Cores redundantly compute LN1 + K/V projections for the full batch element
(so attention needs no cross-core communication); Q / attention / proj / MLP
run only on the core's own 512 rows.  No collectives.

Host-side prep:
  - all weights pre-transposed to [in_feat, out_feat] for contiguous DMA
  - LN gammas folded into the following matmul weights, betas into biases
  - q scale (1/sqrt(hd)) folded into W_q / b_q
  - k bias dropped (softmax shift-invariant), v bias folded into proj bias
    (softmax rows sum to 1)
  - per-core x rows permuted to [own 512 | other 512]; softmax is invariant
    to key/value ordering so attention over permuted K/V is exact.
"""

import sys

if "/opt/trn_rl_repo" not in sys.path:
    sys.path.insert(0, "/opt/trn_rl_repo")

import numpy as np

import concourse.bacc as bacc
import concourse.bass as bass
import concourse.mybir as mybir
import concourse.tile as tile
from concourse.masks import make_identity
from concourse.tile_rust import add_dep_helper

FP = mybir.dt.float32
FPR = mybir.dt.float32r  # fast fp32 matmul mode on trn2
AF = mybir.ActivationFunctionType

N_CORES = 8
P = 128
C = 1024            # model dim
H = 16              # heads
HD = 64             # head dim
HID = 4096          # mlp hidden
N_ALL = 1024        # rows per batch element
N_OWN = 512         # rows owned per core
EPS = 1e-5

CT = C // P         # 8 feature chunks of 128
NT_ALL = N_ALL // P # 8 row tiles
NT_OWN = N_OWN // P # 4 row tiles


def _bcast(ap, p=P):
    """Partition-broadcast a [*free] AP to [p, *free] (step-0 partition dim)."""
    return bass.AP(tensor=ap.tensor, offset=ap.offset, ap=[[0, p], *ap.ap])


def _mm(nc, out, lhsT, rhs, start, stop):
    nc.tensor.matmul(out, lhsT, rhs, start=start, stop=stop)


def _layernorm_tile(nc, pool, out, xt, width, eps_sb):
    """out = (xt - mean(xt)) * rsqrt(var(xt) + eps), rowwise over `width`."""
    ng = width // 512
    st = pool.tile([P, ng, 6], FP, tag="ln_st", name="ln_st")
    for g in range(ng):
        nc.vector.bn_stats(out=st[:, g, :], in_=xt[:, 512 * g:512 * (g + 1)])
    mv = pool.tile([P, 2], FP, tag="ln_mv", name="ln_mv")
    nc.vector.bn_aggr(out=mv, in_=st)
    rstd = pool.tile([P, 1], FP, tag="ln_rstd", name="ln_rstd")
    nc.scalar.activation(out=rstd, in_=mv[:, 1:2], func=AF.Sqrt, bias=eps_sb, scale=1.0)
    nc.vector.reciprocal(out=rstd, in_=rstd)
    for g in range(ng):
        sl = slice(512 * g, 512 * (g + 1))
        nc.vector.tensor_scalar(
            out=out[:, sl], in0=xt[:, sl],
            scalar1=mv[:, 0:1], scalar2=rstd,
            op0=mybir.AluOpType.subtract, op1=mybir.AluOpType.mult,
        )


def build():
    nc = bacc.Bacc("TRN2", target_bir_lowering=False, debug=False,
                   num_devices=N_CORES)

    x_d = nc.dram_tensor("x", [N_ALL, C], FP, kind="ExternalInput")
    wqk_d = nc.dram_tensor("wqk", [C, 2 * C], FP, kind="ExternalInput")
    wv_d = nc.dram_tensor("wv", [C, C], FP, kind="ExternalInput")
    bq_d = nc.dram_tensor("bq", [C], FP, kind="ExternalInput")
    wp_d = nc.dram_tensor("wp", [C, C], FP, kind="ExternalInput")
    bp_d = nc.dram_tensor("bp", [C], FP, kind="ExternalInput")
    w2_d = nc.dram_tensor("w2", [C, HID], FP, kind="ExternalInput")
    b2_d = nc.dram_tensor("b2", [HID], FP, kind="ExternalInput")
    wf2_d = nc.dram_tensor("wf2", [HID, C], FP, kind="ExternalInput")
    bf2_d = nc.dram_tensor("bf2", [C], FP, kind="ExternalInput")
    out_d = nc.dram_tensor("out", [N_OWN, C], FP, kind="ExternalOutput")

    with tile.TileContext(nc, pool_alloc_mode="queue") as tc:
        consts = tc.alloc_tile_pool(name="consts", bufs=1)
        ident = consts.tile([P, P], FP)
        make_identity(nc, ident)
        bq_sb = consts.tile([P, CT], FP)
        nc.sync.dma_start(out=bq_sb, in_=bq_d[:].rearrange("(j p) -> p j", p=P))
        b2_sb = consts.tile([P, HID // P], FP)
        nc.sync.dma_start(out=b2_sb, in_=b2_d[:].rearrange("(j p) -> p j", p=P))
        bp_bc = consts.tile([P, C], FP)
        nc.sync.dma_start(out=bp_bc, in_=_bcast(bp_d[:]))
        bf2_bc = consts.tile([P, C], FP)
        nc.sync.dma_start(out=bf2_bc, in_=_bcast(bf2_d[:]))
        eps_sb = consts.tile([P, 1], FP)
        nc.vector.memset(eps_sb, EPS)
        ones_row = consts.tile([1, HD], FPR)
        nc.vector.memset(ones_row.bitcast(FP), 1.0)

        # ---- Phase A: load x, LN1, transpose h -> hT [C, N_ALL] ----
        hT_pool = tc.alloc_tile_pool(name="hT", bufs=CT)
        pa = tc.alloc_tile_pool(name="pa", bufs=6)
        ha = tc.alloc_tile_pool(name="ha", bufs=3)
        psA = tc.alloc_tile_pool(name="psA", bufs=6, space="PSUM")
        hT = [hT_pool.tile([P, N_ALL], FPR, tag="hT", name="hT") for _ in range(CT)]
        x_tiles = []
        x_dmas = []
        for i in range(NT_ALL):
            xt = pa.tile([P, C], FP, tag="x_in", name="x_in")
            for hh in range(2):
                d = nc.gpsimd.dma_start(
                    out=xt[64 * hh:64 * (hh + 1), :],
                    in_=x_d[P * i + 64 * hh:P * i + 64 * (hh + 1), :])
                x_dmas.append(d)
            x_tiles.append(xt)
        for i in range(NT_ALL):
            xt = x_tiles[i]
            ht = ha.tile([P, C], FP, tag="h", name="h")
            _layernorm_tile(nc, pa, ht, xt, C, eps_sb)
            for j in range(CT):
                ps = psA.tile([P, P], FP, tag="psA", name="psA")
                nc.tensor.transpose(ps, ht[:, P * j:P * (j + 1)], ident)
                if j % 2 == 0:
                    nc.vector.tensor_copy(out=hT[j][:, P * i:P * (i + 1)], in_=ps)
                else:
                    nc.scalar.activation(out=hT[j][:, P * i:P * (i + 1)],
                                         in_=ps, func=AF.Copy)
        ha.release()
        pa.release()
        psA.release()

        # ---- Phase B+C: v projection, then per-head-pair qk + attention ----
        v_pool = tc.alloc_tile_pool(name="v", bufs=NT_ALL)
        wV = tc.alloc_tile_pool(name="wV", bufs=16)
        psV = tc.alloc_tile_pool(name="psV", bufs=6, space="PSUM")
        v_aug = [v_pool.tile([P, H, HD + 1], FPR, tag="v", name="v") for _ in range(NT_ALL)]

        for grp in range(2):
            ws = [wV.tile([P, 512], FPR, tag="wV", name="wV") for _ in range(CT)]
            for c in range(CT):
                d = nc.sync.dma_start(
                    out=ws[c],
                    in_=wv_d[P * c:P * (c + 1), 512 * grp:512 * (grp + 1)].bitcast(FPR))
                if grp == 0:
                    add_dep_helper(d.ins, x_dmas[-1].ins, sync=True,
                                   reason="defer weight stream behind x load")
            for m in range(NT_ALL):
                ps = psV.tile([P, 512], FP, tag="psV", name="psV")
                for c in range(CT):
                    _mm(nc, ps, hT[c][:, P * m:P * (m + 1)], ws[c],
                        c == 0, c == CT - 1)
                h0 = grp * 8
                nc.scalar.activation(
                    out=v_aug[m][:, h0:h0 + 8, 0:HD],
                    in_=ps.rearrange("p (h d) -> p h d", h=8), func=AF.Copy)
        for m in range(NT_ALL):
            nc.vector.memset(v_aug[m][:, :, HD:HD + 1].bitcast(FP), 1.0)
        wV.release()
        psV.release()

        # right stack: attention outputs (live until proj)
        oT_pool = tc.alloc_tile_pool(name="oT", bufs=CT, side="right")
        den_pool = tc.alloc_tile_pool(name="den", bufs=3, side="right")
        oT = [oT_pool.tile([P, N_OWN], FPR, tag="oT", name="oT") for _ in range(CT)]

        wqk = tc.alloc_tile_pool(name="wqk", bufs=12)
        qT_pool = tc.alloc_tile_pool(name="qT", bufs=3)
        kT_pool = tc.alloc_tile_pool(name="kT", bufs=2)
        pt_pool = tc.alloc_tile_pool(name="pt", bufs=9)
        den_row = tc.alloc_tile_pool(name="den_row", bufs=4)
        den_dram = tc.alloc_tile_pool(name="den_dram", bufs=H, space="DRAM")
        psS = tc.alloc_tile_pool(name="psS", bufs=3, space="PSUM")
        psO = tc.alloc_tile_pool(name="psO", bufs=2, space="PSUM")

        for p in range(CT):            # head pairs
            den_p = den_pool.tile([P, N_OWN], FP, tag="den", name="den")
            ws = [wqk.tile([P, 256], FPR, tag="wqk", name="wqk") for _ in range(CT)]
            for c in range(CT):
                d = nc.sync.dma_start(
                    out=ws[c],
                    in_=wqk_d[P * c:P * (c + 1), 256 * p:256 * (p + 1)].bitcast(FPR))
                if p == 0:
                    add_dep_helper(d.ins, x_dmas[-1].ins, sync=True,
                                   reason="defer wqk stream behind x load")
            qTp = qT_pool.tile([P, N_OWN], FPR, tag="qT", name="qT")
            kTp = kT_pool.tile([P, N_ALL], FPR, tag="kT", name="kT")
            ps = psS.tile([P, 1024], FP, tag="psS", name="psS")
            for c in range(CT):
                _mm(nc, ps[:, 0:512], ws[c][:, 0:P], hT[c][:, 0:N_OWN],
                    c == 0, c == CT - 1)
            nc.vector.tensor_scalar_add(out=qTp, in0=ps[:, 0:512],
                                        scalar1=bq_sb[:, p:p + 1])
            for s in range(2):
                ps = psS.tile([P, 1024], FP, tag="psS", name="psS")
                for c in range(CT):
                    _mm(nc, ps[:, 512 * s:512 * (s + 1)], ws[c][:, P:256],
                        hT[c][:, 512 * s:512 * (s + 1)], c == 0, c == CT - 1)
                nc.scalar.activation(out=kTp[:, 512 * s:512 * (s + 1)],
                                     in_=ps[:, 512 * s:512 * (s + 1)], func=AF.Copy)

            for odd in range(2):
                h = 2 * p + odd
                kt = kTp[HD * odd:HD * (odd + 1), :]
                qt = qTp[HD * odd:HD * (odd + 1), :]
                pts = []
                for t in range(4):
                    ps = psS.tile([P, 1024], FP, tag="psS", name="psS")
                    _mm(nc, ps[:, 0:512], kt[:, P * 2 * t:P * (2 * t + 1)], qt,
                        True, True)
                    _mm(nc, ps[:, 512:1024], kt[:, P * (2 * t + 1):P * (2 * t + 2)],
                        qt, True, True)
                    pt = pt_pool.tile([P, 1024], FPR, tag="pt", name="pt")
                    nc.scalar.activation(out=pt, in_=ps, func=AF.Exp)
                    pts.append(pt)
                po = psO.tile([HD + 1, N_OWN], FP, tag="psO", name="psO")
                for t in range(4):
                    _mm(nc, po, v_aug[2 * t][:, h, :], pts[t][:, 0:512],
                        t == 0, False)
                    _mm(nc, po, v_aug[2 * t + 1][:, h, :], pts[t][:, 512:1024],
                        False, t == 3)
                half = slice(HD * odd, HD * (odd + 1))
                nc.vector.tensor_copy(out=oT[p][half, :], in_=po[0:HD, :])
                dr = den_row.tile([1, N_OWN], FPR, tag="denrow", name="denrow")
                nc.vector.tensor_copy(out=dr, in_=po[HD:HD + 1, :])
                if p == CT - 1:
                    pb = psS.tile([HD, N_OWN], FP, tag="psS", name="psS_bc")
                    nc.tensor.matmul(pb, ones_row, dr, start=True, stop=True)
                    nc.vector.reciprocal(out=den_p[half, :], in_=pb[0:HD, :])
                else:
                    dd = den_dram.tile([1, N_OWN], FP, tag="dendram", name="dendram")
                    nc.sync.dma_start(out=dd, in_=dr.bitcast(FP))
                    nc.sync.dma_start(out=den_p[half, :], in_=_bcast(dd[0, :], HD))
                    nc.vector.reciprocal(out=den_p[half, :], in_=den_p[half, :])
                nc.vector.tensor_mul(out=oT[p][half, :], in0=oT[p][half, :],
                                     in1=den_p[half, :])
        den_row.release()
        pt_pool.release()
        kT_pool.release()
        qT_pool.release()
        wqk.release()
        v_pool.release()
        hT_pool.release()
        den_dram.release()
        psO.release()
        psS.release()

        # ---- Phase D+E: proj + residual -> x2; LN2 + transpose -> h2T ----
        x2_pool = tc.alloc_tile_pool(name="x2", bufs=NT_OWN)
        h2T_pool = tc.alloc_tile_pool(name="h2T", bufs=CT)
        wD = tc.alloc_tile_pool(name="wD", bufs=CT)
        xres = tc.alloc_tile_pool(name="xres", bufs=NT_OWN)
        pe = tc.alloc_tile_pool(name="pe", bufs=4)
        he = tc.alloc_tile_pool(name="he", bufs=2)
        psD = tc.alloc_tile_pool(name="psD", bufs=4, space="PSUM")
        psE = tc.alloc_tile_pool(name="psE", bufs=4, space="PSUM")
        h2T = [h2T_pool.tile([P, N_OWN], FPR, tag="h2T", name="h2T") for _ in range(CT)]
        wp_t = [wD.tile([P, C], FPR, tag="wD", name="wD") for _ in range(CT)]
        for c in range(CT):
            nc.sync.dma_start(out=wp_t[c], in_=wp_d[P * c:P * (c + 1), :].bitcast(FPR))
        xr = [xres.tile([P, C], FP, tag="xres", name="xres") for _ in range(NT_OWN)]
        for n in range(NT_OWN):
            nc.sync.dma_start(out=xr[n], in_=x_d[P * n:P * (n + 1), :])
        x2 = [x2_pool.tile([P, C], FP, tag="x2", name="x2") for _ in range(NT_OWN)]
        for n in range(NT_OWN):
            pss = [psD.tile([P, 512], FP, tag="psD", name="psD") for _ in range(2)]
            for of in range(CT):
                for cc in range(2):
                    _mm(nc, pss[cc], oT[of][:, P * n:P * (n + 1)],
                        wp_t[of][:, 512 * cc:512 * (cc + 1)],
                        of == 0, of == CT - 1)
            for cc in range(2):
                sl = slice(512 * cc, 512 * (cc + 1))
                nc.vector.tensor_add(out=x2[n][:, sl], in0=pss[cc], in1=xr[n][:, sl])
                nc.vector.tensor_add(out=x2[n][:, sl], in0=x2[n][:, sl],
                                     in1=bp_bc[:, sl])
            ht = he.tile([P, C], FP, tag="h2", name="h2")
            _layernorm_tile(nc, pe, ht, x2[n], C, eps_sb)
            for j in range(CT):
                ps = psE.tile([P, P], FP, tag="psE", name="psE")
                nc.tensor.transpose(ps, ht[:, P * j:P * (j + 1)], ident)
                if j % 2 == 0:
                    nc.vector.tensor_copy(out=h2T[j][:, P * n:P * (n + 1)], in_=ps)
                else:
                    nc.scalar.activation(out=h2T[j][:, P * n:P * (n + 1)],
                                         in_=ps, func=AF.Copy)
        he.release()
        pe.release()
        xres.release()
        wD.release()
        den_pool.release()
        oT_pool.release()
        psE.release()
        psD.release()

        # ---- Phase F: fc1 + gelu -> h3T [HID, N_OWN] ----
        h3T_pool = tc.alloc_tile_pool(name="h3T", bufs=HID // P, side="right")
        wF = tc.alloc_tile_pool(name="wF", bufs=24)
        psF = tc.alloc_tile_pool(name="psF", bufs=4, space="PSUM")
        h3T = [h3T_pool.tile([P, N_OWN], FPR, tag="h3T", name="h3T") for _ in range(HID // P)]
        for g in range(8):             # groups of 4 hf-tiles
            ws = [wF.tile([P, 512], FPR, tag="wF", name="wF") for _ in range(CT)]
            for c in range(CT):
                nc.sync.dma_start(
                    out=ws[c], in_=w2_d[P * c:P * (c + 1), 512 * g:512 * (g + 1)].bitcast(FPR))
            for f in range(4):
                hf = 4 * g + f
                ps = psF.tile([P, 512], FP, tag="psF", name="psF")
                for c in range(CT):
                    _mm(nc, ps, ws[c][:, P * f:P * (f + 1)], h2T[c],
                        c == 0, c == CT - 1)
                nc.scalar.activation(out=h3T[hf], in_=ps, func=AF.Gelu,
                                     bias=b2_sb[:, hf:hf + 1], scale=1.0)
        wF.release()
        h2T_pool.release()
        psF.release()

        # ---- Phase G: fc2 + residual -> out ----
        wG = tc.alloc_tile_pool(name="wG", bufs=6)
        psG = tc.alloc_tile_pool(name="psG", bufs=1, space="PSUM")
        out_pool = tc.alloc_tile_pool(name="outp", bufs=NT_OWN)
        pg = [[psG.tile([P, 512], FP, tag=f"psG{n}_{cc}", name=f"psG{n}_{cc}") for cc in range(2)]
              for n in range(NT_OWN)]
        for hf in range(HID // P):
            wt = wG.tile([P, C], FPR, tag="wG", name="wG")
            nc.sync.dma_start(out=wt, in_=wf2_d[P * hf:P * (hf + 1), :].bitcast(FPR))
            for n in range(NT_OWN):
                for cc in range(2):
                    _mm(nc, pg[n][cc], h3T[hf][:, P * n:P * (n + 1)],
                        wt[:, 512 * cc:512 * (cc + 1)],
                        hf == 0, hf == HID // P - 1)
        for n in range(NT_OWN):
            x3 = out_pool.tile([P, C], FP, tag="x3", name="x3")
            for cc in range(2):
                sl = slice(512 * cc, 512 * (cc + 1))
                nc.vector.tensor_add(out=x3[:, sl], in0=pg[n][cc], in1=x2[n][:, sl])
                nc.vector.tensor_add(out=x3[:, sl], in0=x3[:, sl], in1=bf2_bc[:, sl])
            nc.sync.dma_start(out=out_d[P * n:P * (n + 1), :], in_=x3)
        out_pool.release()
        wG.release()
        h3T_pool.release()
        x2_pool.release()
        psG.release()
        consts.release()

    nc.compile()
    return nc


_NC = None


def _get_nc():
    global _NC
    if _NC is None:
        _NC = build()
    return _NC


def _prep(inputs):
    f32 = lambda a: np.ascontiguousarray(np.asarray(a, dtype=np.float32))
    x = f32(inputs["x"])
    qkv_w, qkv_b = f32(inputs["qkv_w"]), f32(inputs["qkv_b"])
    proj_w, proj_b = f32(inputs["proj_w"]), f32(inputs["proj_b"])
    fc1_w, fc1_b = f32(inputs["fc1_w"]), f32(inputs["fc1_b"])
    fc2_w, fc2_b = f32(inputs["fc2_w"]), f32(inputs["fc2_b"])
    ln1_g, ln1_b = f32(inputs["ln1_g"]), f32(inputs["ln1_b"])
    ln2_g, ln2_b = f32(inputs["ln2_g"]), f32(inputs["ln2_b"])

    scale = np.float32(HD ** -0.5)
    w1 = (qkv_w * ln1_g[None, :]).T                 # [C, 3C]
    b1 = qkv_b + qkv_w @ ln1_b                      # [3C]
    wq = w1[:, :C] * scale
    wk = w1[:, C:2 * C]
    wv = np.ascontiguousarray(w1[:, 2 * C:])
    # pair-interleave q and k 128-col tiles: [q_p | k_p] per head pair p
    wqk = np.empty((C, 2 * C), dtype=np.float32)
    for p_ in range(C // P):
        wqk[:, 256 * p_:256 * p_ + P] = wq[:, P * p_:P * (p_ + 1)]
        wqk[:, 256 * p_ + P:256 * (p_ + 1)] = wk[:, P * p_:P * (p_ + 1)]
    bq = (b1[:C] * scale).copy()
    bv = b1[2 * C:]
    wp = proj_w.T.copy()                            # [C, C]
    bp = proj_b + proj_w @ bv
    w2 = (fc1_w * ln2_g[None, :]).T.copy()          # [C, HID]
    b2 = fc1_b + fc1_w @ ln2_b
    wf2 = fc2_w.T.copy()                            # [HID, C]
    bf2 = fc2_b

    shared = dict(wqk=f32(wqk), wv=f32(wv), bq=f32(bq), wp=f32(wp), bp=f32(bp),
                  w2=f32(w2), b2=f32(b2), wf2=f32(wf2), bf2=f32(bf2))
    in_maps = []
    for c in range(N_CORES):
        b, half = divmod(c, 2)
        own = x[b, N_OWN * half:N_OWN * (half + 1), :]
        oth = x[b, N_OWN * (1 - half):N_OWN * (2 - half), :]
        xp = np.concatenate([own, oth], axis=0)
        in_maps.append({"x": xp, **shared})
    return in_maps


def run(inputs, trace=False, trace_kwargs=None):
    from concourse.bass_utils import run_bass_kernel_spmd
    nc = _get_nc()
    in_maps = _prep(inputs)
    res = run_bass_kernel_spmd(nc, in_maps, core_ids=list(range(N_CORES)),
                               trace=trace, **(trace_kwargs or {}))
    B = 4
    out = np.empty((B, N_ALL, C), dtype=np.float32)
    for c in range(N_CORES):
        b, half = divmod(c, 2)
        out[b, N_OWN * half:N_OWN * (half + 1), :] = res.results[c]["out"]
    return out, res


def kernel(**inputs):
    out, _ = run(inputs, trace=False)
    return out



# revision 13
# speedup vs baseline: 1.0735x; 1.0735x over previous
"""Dense transformer block on 8 Trainium2 NeuronCores — fp8 DoubleRow edition.

Sharding: each core owns half a batch element (512 rows out of [4, 1024, C]).
Cores redundantly compute LN1 + K/V projections for the full batch element
(so attention needs no cross-core communication); Q / attention / proj / MLP
run only on the core's own 512 rows.  No collectives.

Precision plan (rel-l2 budget 2e-2; this lands ~6e-3):
  - x streamed as bf16 for the LN1 path; fp32 rows reloaded for the residual.
  - attention-side matmuls (q/k/v proj, attn@V, proj) in fp8e4m3 DoubleRow
    (2 k-tiles per pass, 0.5 PE cycles/row): activations scaled x32, weights
    x1024 (q-weights x8192 with softmax scale folded), compensated in the
    PSUM->SBUF copies.  q/k kept fp32r; scores matmul fp32r; exp output
    quantized to fp8 (exp(s)/4 to dodge e4m3 saturation at 240) — softmax
    renormalization by the appended ones-column cancels P-quantization scale.
  - MLP (fc1/gelu/fc2) in bf16 end-to-end: same PE rate as fp32r, half DMA.

Host-side prep:
  - weights pre-transposed to [in_feat, out_feat], LN gammas folded into the
    following matmul weights, betas into biases; q scale folded into W_q/b_q;
    k bias dropped (softmax shift-invariant), v bias folded into proj bias;
    proj bias folded into the residual rows (xrb = x_own + bp).
  - fp8 weights packed in DoubleRow plane-pair layout [128, k-pair, 2, out].
  - per-core x rows permuted to [own 512 | other 512]; softmax is invariant
    to key/value ordering so attention over permuted K/V is exact.
"""

import sys

if "/opt/trn_rl_repo" not in sys.path:
    sys.path.insert(0, "/opt/trn_rl_repo")

import numpy as np
import ml_dtypes

import concourse.bacc as bacc
import concourse.bass as bass
import concourse.mybir as mybir
import concourse.tile as tile
from concourse.masks import make_identity

FP = mybir.dt.float32
FPR = mybir.dt.float32r
BF = mybir.dt.bfloat16
F8 = mybir.dt.float8e4
AF = mybir.ActivationFunctionType
DR = mybir.MatmulPerfMode.DoubleRow

NP_BF = ml_dtypes.bfloat16
NP_F8 = ml_dtypes.float8_e4m3

N_CORES = 8
P = 128
C = 1024            # model dim
H = 16              # heads
HD = 64             # head dim
HID = 4096          # mlp hidden
N_ALL = 1024        # rows per batch element
N_OWN = 512         # rows owned per core
EPS = 1e-5

CT = C // P         # 8 feature chunks of 128
CJ = CT // 2        # 4 DoubleRow plane-pairs
NT_ALL = N_ALL // P # 8 row tiles
NT_OWN = N_OWN // P # 4 row tiles

# quantization scales (powers of two; compensated in PSUM->SBUF copies)
SH = 32.0           # fp8 activation scale (h, v, o)
SW = 1024.0         # fp8 weight scale (wk, wv, wp)
SWQ = 8192.0        # fp8 q-weight scale (softmax 1/8 folded in first)
LN2_2 = float(2.0 * np.log(2.0))


def _bcast(ap, p=P):
    """Partition-broadcast a [*free] AP to [p, *free] (step-0 partition dim)."""
    return bass.AP(tensor=ap.tensor, offset=ap.offset, ap=[[0, p], *ap.ap])


def _layernorm_tile(nc, pool, out, xt, eps_sb, sqrt_scale):
    """out = (xt - mean) * (1/sqrt_scale) / sqrt(var + eps), rowwise over C.

    eps_sb must hold EPS * sqrt_scale^2 so that
    rstd = 1 / sqrt(sqrt_scale^2 * var + eps * sqrt_scale^2)
         = (1/sqrt_scale) / sqrt(var + eps).
    Used with sqrt_scale=2^-5 to get an extra x32 folded into the output.
    """
    st = pool.tile([P, 2, 6], FP, tag="ln_st", name="ln_st")
    for g in range(2):
        nc.vector.bn_stats(out=st[:, g, :], in_=xt[:, 512 * g:512 * (g + 1)])
    mv = pool.tile([P, 2], FP, tag="ln_mv", name="ln_mv")
    nc.vector.bn_aggr(out=mv, in_=st)
    rstd = pool.tile([P, 1], FP, tag="ln_rstd", name="ln_rstd")
    nc.scalar.activation(out=rstd, in_=mv[:, 1:2], func=AF.Sqrt, bias=eps_sb,
                         scale=sqrt_scale * sqrt_scale)
    nc.vector.reciprocal(out=rstd, in_=rstd)
    for g in range(2):
        sl = slice(512 * g, 512 * (g + 1))
        nc.vector.tensor_scalar(
            out=out[:, sl], in0=xt[:, sl],
            scalar1=mv[:, 0:1], scalar2=rstd,
            op0=mybir.AluOpType.subtract, op1=mybir.AluOpType.mult,
        )


def build():
    nc = bacc.Bacc("TRN2", target_bir_lowering=False, debug=False,
                   num_devices=N_CORES)

    x_d = nc.dram_tensor("x", [N_ALL, C], BF, kind="ExternalInput")
    xrb_d = nc.dram_tensor("xrb", [N_OWN, C], FP, kind="ExternalInput")
    wqk_d = nc.dram_tensor("wqk", [CT, P, CJ, 2, 256], F8, kind="ExternalInput")
    wv_d = nc.dram_tensor("wv", [2, P, CJ, 2, 512], F8, kind="ExternalInput")
    bq_d = nc.dram_tensor("bq", [C], FP, kind="ExternalInput")
    wp_d = nc.dram_tensor("wp", [P, CJ, 2, C], F8, kind="ExternalInput")
    w1_d = nc.dram_tensor("w1", [8, P, CT, 512], BF, kind="ExternalInput")
    b2_d = nc.dram_tensor("b2", [HID], FP, kind="ExternalInput")
    wf2_d = nc.dram_tensor("wf2", [HID // P, P, C], BF, kind="ExternalInput")
    bf2_d = nc.dram_tensor("bf2", [C], FP, kind="ExternalInput")
    out_d = nc.dram_tensor("out", [N_OWN, C], FP, kind="ExternalOutput")

    with tile.TileContext(nc, pool_alloc_mode="queue") as tc:
        consts = tc.alloc_tile_pool(name="consts", bufs=1)
        identb = consts.tile([P, P], BF)
        make_identity(nc, identb)
        bq_sb = consts.tile([P, CT], FP)
        nc.sync.dma_start(out=bq_sb, in_=bq_d[:].rearrange("(j p) -> p j", p=P))
        b2_sb = consts.tile([P, HID // P], FP)
        nc.sync.dma_start(out=b2_sb, in_=b2_d[:].rearrange("(j p) -> p j", p=P))
        bf2_bc = consts.tile([P, C], FP)
        nc.sync.dma_start(out=bf2_bc, in_=_bcast(bf2_d[:]))
        eps1_sb = consts.tile([P, 1], FP)       # for LN1: eps * (2^-5)^2
        nc.vector.memset(eps1_sb, EPS / (SH * SH))
        eps2_sb = consts.tile([P, 1], FP)       # for LN2: plain eps
        nc.vector.memset(eps2_sb, EPS)
        nexp_b = consts.tile([P, 1], FP)        # exp bias: -2ln2 -> exp(s)/4
        nc.vector.memset(nexp_b, -LN2_2)
        ones_row = consts.tile([1, HD], FPR)
        nc.vector.memset(ones_row.bitcast(FP), 1.0)

        # fc2 weights: fully resident, prefetched from the start on the
        # scalar engine's DMA queue so they never block later streams.
        wf2_pool = tc.alloc_tile_pool(name="wf2", bufs=HID // P, side="right")
        wf2_t = [wf2_pool.tile([P, C], BF, tag="wf2", name="wf2")
                 for _ in range(HID // P)]
        for hf in range(HID // P):
            nc.scalar.dma_start(out=wf2_t[hf], in_=wf2_d[hf])

        # Long-lived pools allocated up-front in reverse order of death so
        # every release happens at the top of the LIFO pool stack:
        # x2 dies at phase G end, h2T at F end, wD/xres at D end.
        x2_pool = tc.alloc_tile_pool(name="x2", bufs=NT_OWN)
        h2T_pool = tc.alloc_tile_pool(name="h2T", bufs=CT)
        wD = tc.alloc_tile_pool(name="wD", bufs=1)
        xres = tc.alloc_tile_pool(name="xres", bufs=NT_OWN)

        # ---- Phase A: load x (bf16), LN1 -> h fp8 (x32), transpose ----
        h8_pool = tc.alloc_tile_pool(name="h8", bufs=CJ)
        pa = tc.alloc_tile_pool(name="pa", bufs=6)
        ha = tc.alloc_tile_pool(name="ha", bufs=3)
        psA = tc.alloc_tile_pool(name="psA", bufs=6, space="PSUM")
        h8 = [h8_pool.tile([P, 2, N_ALL], F8, tag="h8", name="h8")
              for _ in range(CJ)]
        x_tiles = []
        for i in range(NT_ALL):
            xt = pa.tile([P, C], BF, tag="x_in", name="x_in")
            for hh in range(2):
                nc.gpsimd.dma_start(
                    out=xt[64 * hh:64 * (hh + 1), :],
                    in_=x_d[P * i + 64 * hh:P * i + 64 * (hh + 1), :])
            x_tiles.append(xt)
        for i in range(NT_ALL):
            ht = ha.tile([P, C], BF, tag="h", name="h")
            _layernorm_tile(nc, pa, ht, x_tiles[i], eps1_sb, 1.0 / SH)
            for j in range(CT):
                ps = psA.tile([P, P], BF, tag="psA", name="psA")
                nc.tensor.transpose(ps, ht[:, P * j:P * (j + 1)], identb)
                dst = h8[j // 2][:, j % 2, P * i:P * (i + 1)]
                if j % 2 == 0:
                    nc.vector.tensor_copy(out=dst, in_=ps)
                else:
                    nc.scalar.activation(out=dst, in_=ps, func=AF.Copy)
        ha.release()
        pa.release()
        psA.release()

        # ---- Phase B: V projection -> v8 (paired m-planes, fp8 x32) ----
        v_pool = tc.alloc_tile_pool(name="v8", bufs=NT_ALL // 2)
        wV = tc.alloc_tile_pool(name="wV", bufs=2)
        psV = tc.alloc_tile_pool(name="psV", bufs=3, space="PSUM")
        v8 = [v_pool.tile([P, 2, H, HD + 1], F8, tag="v8", name="v8")
              for _ in range(NT_ALL // 2)]
        for grp in range(2):
            wt = wV.tile([P, CJ, 2, 512], F8, tag="wV", name="wV")
            nc.sync.dma_start(out=wt, in_=wv_d[grp])
            for m in range(NT_ALL):
                ps = psV.tile([P, 512], FP, tag="psV", name="psV")
                for j in range(CJ):
                    nc.tensor.matmul(ps, h8[j][:, :, P * m:P * (m + 1)],
                                     wt[:, j, :, :], start=j == 0,
                                     stop=j == CJ - 1, perf_mode=DR)
                nc.scalar.activation(
                    out=v8[m // 2][:, m % 2, 8 * grp:8 * (grp + 1), 0:HD],
                    in_=ps.rearrange("p (h d) -> p h d", h=8), func=AF.Copy,
                    scale=SH / (SH * SW))
        for t in range(NT_ALL // 2):
            nc.vector.memset(v8[t][:, :, :, HD:HD + 1], 1.0)
        wV.release()
        psV.release()

        # right stack: attention outputs (live until proj)
        o8_pool = tc.alloc_tile_pool(name="o8", bufs=CJ, side="right")
        den_pool = tc.alloc_tile_pool(name="den", bufs=3, side="right")
        o8 = [o8_pool.tile([P, 2, N_OWN], F8, tag="o8", name="o8")
              for _ in range(CJ)]

        # prefetched proj weights + residual rows (consumed in Phase D)
        wp_t = wD.tile([P, CJ, 2, C], F8, tag="wD", name="wD")
        nc.sync.dma_start(out=wp_t, in_=wp_d[:])
        xr = [xres.tile([P, C], FP, tag="xres", name="xres")
              for _ in range(NT_OWN)]
        for n in range(NT_OWN):
            nc.gpsimd.dma_start(out=xr[n], in_=xrb_d[P * n:P * (n + 1), :])

        # ---- Phase C: per-head-pair qk + attention ----
        wqk = tc.alloc_tile_pool(name="wqk", bufs=3)
        qT_pool = tc.alloc_tile_pool(name="qT", bufs=3)
        kT_pool = tc.alloc_tile_pool(name="kT", bufs=2)
        pt_pool = tc.alloc_tile_pool(name="pt", bufs=9)
        ot_pool = tc.alloc_tile_pool(name="ot", bufs=2)
        den_row = tc.alloc_tile_pool(name="den_row", bufs=4)
        den_dram = tc.alloc_tile_pool(name="den_dram", bufs=H, space="DRAM")
        psS = tc.alloc_tile_pool(name="psS", bufs=3, space="PSUM")
        psO = tc.alloc_tile_pool(name="psO", bufs=2, space="PSUM")

        for p in range(CT):            # head pairs
            den_p = den_pool.tile([P, N_OWN], FP, tag="den", name="den")
            wt = wqk.tile([P, CJ, 2, 256], F8, tag="wqk", name="wqk")
            nc.sync.dma_start(out=wt, in_=wqk_d[p])
            qTp = qT_pool.tile([P, N_OWN], FPR, tag="qT", name="qT")
            kTp = kT_pool.tile([P, N_ALL], FPR, tag="kT", name="kT")
            otmp = ot_pool.tile([P, N_OWN], FP, tag="ot", name="ot")
            ps = psS.tile([P, 1024], FP, tag="psS", name="psS")
            for j in range(CJ):
                nc.tensor.matmul(ps[:, 0:512], wt[:, j, :, 0:P],
                                 h8[j][:, :, 0:N_OWN], start=j == 0,
                                 stop=j == CJ - 1, perf_mode=DR)
            nc.scalar.activation(out=qTp, in_=ps[:, 0:512], func=AF.Identity,
                                 bias=bq_sb[:, p:p + 1], scale=1.0 / (SH * SWQ))
            for s in range(2):
                ps = psS.tile([P, 1024], FP, tag="psS", name="psS")
                for j in range(CJ):
                    nc.tensor.matmul(ps[:, 512 * s:512 * (s + 1)],
                                     wt[:, j, :, P:256],
                                     h8[j][:, :, 512 * s:512 * (s + 1)],
                                     start=j == 0, stop=j == CJ - 1,
                                     perf_mode=DR)
                nc.scalar.activation(out=kTp[:, 512 * s:512 * (s + 1)],
                                     in_=ps[:, 512 * s:512 * (s + 1)],
                                     func=AF.Copy, scale=1.0 / (SH * SW))

            for odd in range(2):
                h = 2 * p + odd
                kt = kTp[HD * odd:HD * (odd + 1), :]
                qt = qTp[HD * odd:HD * (odd + 1), :]
                pts = []
                for t in range(4):
                    ps = psS.tile([P, 1024], FP, tag="psS", name="psS")
                    nc.tensor.matmul(ps[:, 0:512], kt[:, P * 2 * t:P * (2 * t + 1)],
                                     qt, start=True, stop=True)
                    nc.tensor.matmul(ps[:, 512:1024],
                                     kt[:, P * (2 * t + 1):P * (2 * t + 2)],
                                     qt, start=True, stop=True)
                    pt = pt_pool.tile([P, 2, 512], F8, tag="pt", name="pt")
                    nc.scalar.activation(out=pt, in_=ps.rearrange(
                        "p (a b) -> p a b", a=2), func=AF.Exp, bias=nexp_b,
                        scale=1.0)
                    pts.append(pt)
                po = psO.tile([HD + 1, N_OWN], FP, tag="psO", name="psO")
                for t in range(4):
                    nc.tensor.matmul(po, v8[t][:, :, h, :], pts[t],
                                     start=t == 0, stop=t == 3, perf_mode=DR)
                half = slice(HD * odd, HD * (odd + 1))
                nc.vector.tensor_copy(out=otmp[half, :], in_=po[0:HD, :])
                dr = den_row.tile([1, N_OWN], FPR, tag="denrow", name="denrow")
                nc.vector.tensor_copy(out=dr, in_=po[HD:HD + 1, :])
                if p == CT - 1:
                    pb = psS.tile([HD, N_OWN], FP, tag="psS", name="psS_bc")
                    nc.tensor.matmul(pb, ones_row, dr, start=True, stop=True)
                    nc.vector.reciprocal(out=den_p[half, :], in_=pb[0:HD, :])
                else:
                    dd = den_dram.tile([1, N_OWN], FP, tag="dendram", name="dendram")
                    nc.sync.dma_start(out=dd, in_=dr.bitcast(FP))
                    nc.sync.dma_start(out=den_p[half, :], in_=_bcast(dd[0, :], HD))
                    nc.vector.reciprocal(out=den_p[half, :], in_=den_p[half, :])
                nc.vector.tensor_mul(out=o8[p // 2][half, p % 2, :],
                                     in0=otmp[half, :], in1=den_p[half, :])
        den_row.release()
        ot_pool.release()
        pt_pool.release()
        kT_pool.release()
        qT_pool.release()
        wqk.release()
        v_pool.release()
        h8_pool.release()
        den_dram.release()
        psO.release()
        psS.release()

        # ---- Phase D+E: proj + residual -> x2; LN2 -> h2T (bf16) ----
        pe = tc.alloc_tile_pool(name="pe", bufs=4)
        he = tc.alloc_tile_pool(name="he", bufs=2)
        psD = tc.alloc_tile_pool(name="psD", bufs=4, space="PSUM")
        psE = tc.alloc_tile_pool(name="psE", bufs=4, space="PSUM")
        h2T = [h2T_pool.tile([P, N_OWN], BF, tag="h2T", name="h2T")
               for _ in range(CT)]
        x2 = [x2_pool.tile([P, C], FP, tag="x2", name="x2")
              for _ in range(NT_OWN)]
        for n in range(NT_OWN):
            pss = [psD.tile([P, 512], FP, tag="psD", name="psD") for _ in range(2)]
            for j in range(CJ):
                for cc in range(2):
                    nc.tensor.matmul(pss[cc], o8[j][:, :, P * n:P * (n + 1)],
                                     wp_t[:, j, :, 512 * cc:512 * (cc + 1)],
                                     start=j == 0, stop=j == CJ - 1,
                                     perf_mode=DR)
            for cc in range(2):
                sl = slice(512 * cc, 512 * (cc + 1))
                nc.scalar.activation(out=x2[n][:, sl], in_=pss[cc],
                                     func=AF.Copy, scale=1.0 / (SH * SW))
                nc.vector.tensor_add(out=x2[n][:, sl], in0=x2[n][:, sl],
                                     in1=xr[n][:, sl])
            ht = he.tile([P, C], BF, tag="h2", name="h2")
            _layernorm_tile(nc, pe, ht, x2[n], eps2_sb, 1.0)
            for j in range(CT):
                ps = psE.tile([P, P], BF, tag="psE", name="psE")
                nc.tensor.transpose(ps, ht[:, P * j:P * (j + 1)], identb)
                dst = h2T[j][:, P * n:P * (n + 1)]
                if j % 2 == 0:
                    nc.vector.tensor_copy(out=dst, in_=ps)
                else:
                    nc.scalar.activation(out=dst, in_=ps, func=AF.Copy)
        he.release()
        pe.release()
        xres.release()
        wD.release()
        den_pool.release()
        o8_pool.release()
        psE.release()
        psD.release()

        # ---- Phase F: fc1 + gelu -> h3T bf16 [HID, N_OWN] ----
        h3T_pool = tc.alloc_tile_pool(name="h3T", bufs=HID // P, side="right")
        wF = tc.alloc_tile_pool(name="wF", bufs=3)
        psF = tc.alloc_tile_pool(name="psF", bufs=4, space="PSUM")
        h3T = [h3T_pool.tile([P, N_OWN], BF, tag="h3T", name="h3T")
               for _ in range(HID // P)]
        for g in range(8):             # groups of 4 hf-tiles
            wt = wF.tile([P, CT, 512], BF, tag="wF", name="wF")
            nc.sync.dma_start(out=wt, in_=w1_d[g])
            for f in range(4):
                hf = 4 * g + f
                ps = psF.tile([P, 512], FP, tag="psF", name="psF")
                for c in range(CT):
                    nc.tensor.matmul(ps, wt[:, c, P * f:P * (f + 1)], h2T[c],
                                     start=c == 0, stop=c == CT - 1)
                nc.scalar.activation(out=h3T[hf], in_=ps, func=AF.Gelu,
                                     bias=b2_sb[:, hf:hf + 1], scale=1.0)
        wF.release()
        h2T_pool.release()
        psF.release()

        # ---- Phase G: fc2 + residual -> out (n-outer so tiles finish early) --
        psG = tc.alloc_tile_pool(name="psG", bufs=4, space="PSUM")
        out_pool = tc.alloc_tile_pool(name="outp", bufs=2)
        for n in range(NT_OWN):
            pg = [psG.tile([P, 512], FP, tag="psG", name="psG") for _ in range(2)]
            for hf in range(HID // P):
                for cc in range(2):
                    nc.tensor.matmul(pg[cc], h3T[hf][:, P * n:P * (n + 1)],
                                     wf2_t[hf][:, 512 * cc:512 * (cc + 1)],
                                     start=hf == 0, stop=hf == HID // P - 1)
            x3 = out_pool.tile([P, C], FP, tag="x3", name="x3")
            for cc in range(2):
                sl = slice(512 * cc, 512 * (cc + 1))
                nc.vector.tensor_add(out=x3[:, sl], in0=pg[cc], in1=x2[n][:, sl])
                nc.vector.tensor_add(out=x3[:, sl], in0=x3[:, sl],
                                     in1=bf2_bc[:, sl])
            nc.gpsimd.dma_start(out=out_d[P * n:P * (n + 1), :], in_=x3)
        out_pool.release()
        h3T_pool.release()
        x2_pool.release()
        wf2_pool.release()
        psG.release()
        consts.release()

    nc.compile()
    return nc


_NC = None


def _get_nc():
    global _NC
    if _NC is None:
        _NC = build()
    return _NC


def _q8(a, s):
    return np.clip(np.asarray(a, np.float32) * s, -240.0, 240.0).astype(NP_F8)


def _prep(inputs):
    f32 = lambda a: np.ascontiguousarray(np.asarray(a, dtype=np.float32))
    x = f32(inputs["x"])
    qkv_w, qkv_b = f32(inputs["qkv_w"]), f32(inputs["qkv_b"])
    proj_w, proj_b = f32(inputs["proj_w"]), f32(inputs["proj_b"])
    fc1_w, fc1_b = f32(inputs["fc1_w"]), f32(inputs["fc1_b"])
    fc2_w, fc2_b = f32(inputs["fc2_w"]), f32(inputs["fc2_b"])
    ln1_g, ln1_b = f32(inputs["ln1_g"]), f32(inputs["ln1_b"])
    ln2_g, ln2_b = f32(inputs["ln2_g"]), f32(inputs["ln2_b"])

    scale = np.float32(HD ** -0.5)
    w1 = (qkv_w * ln1_g[None, :]).T                 # [C, 3C]
    b1 = qkv_b + qkv_w @ ln1_b                      # [3C]
    wq = w1[:, :C] * scale
    wk = w1[:, C:2 * C]
    wv = w1[:, 2 * C:]
    # DoubleRow plane-pair packing: [p, kp, j, t, m] = w[(2j+t)*128+kp, col]
    def pack(w, s):
        # w: [C, M] fp32 -> [128, CJ, 2, M] fp8 with k-pairs in planes
        wq_ = _q8(w, s)                             # [C, M]
        return np.ascontiguousarray(
            wq_.reshape(CJ, 2, P, -1).transpose(2, 0, 1, 3))
    wqk8 = np.empty((CT, P, CJ, 2, 256), dtype=NP_F8)
    for p_ in range(CT):
        wqk8[p_, :, :, :, 0:P] = pack(wq[:, P * p_:P * (p_ + 1)], SWQ)
        wqk8[p_, :, :, :, P:256] = pack(wk[:, P * p_:P * (p_ + 1)], SW)
    wv8 = np.empty((2, P, CJ, 2, 512), dtype=NP_F8)
    for g in range(2):
        wv8[g] = pack(wv[:, 512 * g:512 * (g + 1)], SW)
    bq = (b1[:C] * scale).copy()
    bv = b1[2 * C:]
    wp8 = pack(proj_w.T, SW)                        # [128, CJ, 2, C]
    bp = proj_b + proj_w @ bv
    w1w = (fc1_w * ln2_g[None, :]).T                # [C, HID]
    w1b = np.ascontiguousarray(
        w1w.astype(NP_BF).reshape(CT, P, 8, 512).transpose(2, 1, 0, 3))
    b2 = fc1_b + fc1_w @ ln2_b
    wf2b = np.ascontiguousarray(
        fc2_w.T.astype(NP_BF).reshape(HID // P, P, C))
    bf2 = fc2_b

    shared = dict(wqk=wqk8, wv=wv8, bq=f32(bq), wp=wp8,
                  w1=w1b, b2=f32(b2), wf2=wf2b, bf2=f32(bf2))
    in_maps = []
    for c in range(N_CORES):
        b, half = divmod(c, 2)
        own = x[b, N_OWN * half:N_OWN * (half + 1), :]
        oth = x[b, N_OWN * (1 - half):N_OWN * (2 - half), :]
        xp = np.concatenate([own, oth], axis=0)
        xrb = own + bp[None, :]
        in_maps.append({"x": xp.astype(NP_BF), "xrb": f32(xrb), **shared})
    return in_maps


def run(inputs, trace=False, trace_kwargs=None):
    from concourse.bass_utils import run_bass_kernel_spmd
    nc = _get_nc()
    in_maps = _prep(inputs)
    res = run_bass_kernel_spmd(nc, in_maps, core_ids=list(range(N_CORES)),
                               trace=trace, **(trace_kwargs or {}))
    B = 4
    out = np.empty((B, N_ALL, C), dtype=np.float32)
    for c in range(N_CORES):
        b, half = divmod(c, 2)
        out[b, N_OWN * half:N_OWN * (half + 1), :] = res.results[c]["out"]
    return out, res


def kernel(**inputs):
    out, _ = run(inputs, trace=False)
    return out
